# revision 7
# baseline (speedup 1.0000x reference)
"""FLC pooling (FFT2 -> center-crop low freqs -> IFFT2, real part) on 8 trn2 cores.

Math: per (n,c) slice, out = Re(M @ X @ M.T) where M (112x224) is the 1D
fft -> fftshift -> crop -> ifftshift -> ifft operator. Im(M) is exactly
rank-1 (= outer(a, b), a[u] = a0*(-1)^u), so with R = Re(M), G = [R; b]:

    out_ext = G @ X @ G.T            (113x113; [112,112] entry = b'Xb)
    out = out_ext[:112,:112] - out_ext[112,112] * a0^2 * checkerboard

Device pipeline (fp16 operands, fp32 PSUM accumulation):
    W1T = X.T @ G.T      pass 1: stationary = X chunks (fp16), streams G.T;
                         produces the *transposed* intermediate directly,
                         so no PE transposes / identity are needed.
    V   = G @ W1T        pass 2: = out_ext^T, 4 slices batched (N=452)
    s   = b.T X b        tiny matmul against W1T col 112, broadcast to
                         all partitions via a constant-column lhsT
    vout = cneg*s + V    one fused DVE scalar_tensor_tensor per slice
Host unshard transposes each 112x112 slice (free re-layout).

x is loaded by gpsimd casting DMA (fp32 HBM -> fp16 SBUF), keeping the
Sync engine free and halving SBUF traffic; 4 slices per DMA descriptor.

Sharding: batch*channel = 1024 independent (n,c) slices -> 128 per core.
"""

import sys

sys.path.insert(0, "/opt/trn_rl_repo")

import numpy as np

import concourse.bass as bass  # noqa: F401
import concourse.mybir as mybir
import concourse.tile as tile
from concourse import bacc
from concourse.bass_utils import run_bass_kernel_spmd

N = 224
NH = 112
NG = 113  # rows of G = [R; b]
B, C = 16, 64
NCORES = 8
NSLICES = B * C // NCORES  # 128 slices per core
F32 = mybir.dt.float32
F16 = mybir.dt.float16


def _build_consts():
    F = np.fft.fft(np.eye(N), axis=0, norm="forward")
    M = np.fft.ifft(
        np.fft.ifftshift(np.fft.fftshift(F, axes=0)[N // 4 : 3 * N // 4], axes=0),
        axis=0,
        norm="forward",
    )
    R, S = M.real, M.imag
    u, sv, vt = np.linalg.svd(S)
    a = u[:, 0] * np.sqrt(sv[0])
    b = vt[0] * np.sqrt(sv[0])
    if np.abs(S - np.outer(a, b)).max() > 1e-10:
        a, b = -a, -b
    assert np.abs(S - np.outer(a, b)).max() < 1e-12
    G = np.vstack([R, b[None, :]])  # [113, 224]
    # gt16[c][i, u] = G[u, 112c + i]  (G^T row chunks, fp16; pass-2 lhsT)
    gt16 = np.ascontiguousarray(G.T.reshape(2, NH, NG)).astype(np.float16)
    # gtp16[e][p, u] = G[u, 2p + e]  (G^T rows by parity, fp16; pass-1 rhs --
    # pairs with x loaded two-adjacent-rows-per-partition)
    gtp16 = np.ascontiguousarray(
        G.T.reshape(NH, 2, NG).transpose(1, 0, 2)
    ).astype(np.float16)
    # bbc16[c][j, m] = b[112c + j] for all m (column-broadcast b)
    bbc16 = np.ascontiguousarray(
        np.repeat(b.reshape(2, NH, 1), NH, axis=2)
    ).astype(np.float16)
    a0sq = float(a[0] * a[0])  # = 1/224
    vv = np.arange(NH)
    cneg = (-a0sq * ((-1.0) ** (vv[:, None] + vv[None, :]))).astype(np.float32)
    return gt16, gtp16, bbc16, cneg


def _build_nc():
    nc = bacc.Bacc("TRN2", target_bir_lowering=False, debug=False)
    x = nc.dram_tensor("x", [NSLICES, N, N], F32, kind="ExternalInput").ap()
    gt = nc.dram_tensor("gt", [2, NH, NG], F16, kind="ExternalInput").ap()
    gtp = nc.dram_tensor("gtp", [2, NH, NG], F16, kind="ExternalInput").ap()
    bbc = nc.dram_tensor("bbc", [2, NH, NH], F16, kind="ExternalInput").ap()
    cneg = nc.dram_tensor("cneg", [NH, NH], F32, kind="ExternalInput").ap()
    outT = nc.dram_tensor("outT", [NSLICES, NH, NH], F32, kind="ExternalOutput").ap()

    mult = mybir.AluOpType.mult
    add = mybir.AluOpType.add

    with tile.TileContext(nc) as tc:
        with (
            tc.tile_pool(name="consts", bufs=1) as cpool,
            tc.tile_pool(name="xt", bufs=3) as xpool,
            tc.tile_pool(name="w1t4", bufs=3) as w1t4_pool,
            tc.tile_pool(name="vout", bufs=3) as vout_pool,
            tc.tile_pool(name="w1tp", bufs=3, space="PSUM") as w1tpsum,
            tc.tile_pool(name="v4p", bufs=2, space="PSUM") as vpsum,
            tc.tile_pool(name="s4p", bufs=2, space="PSUM") as spsum,
        ):
            gt_sb = cpool.tile([NH, 2, NG], F16)
            nc.sync.dma_start(gt_sb[:], gt.rearrange("c i u -> i c u"))
            gtp_sb = cpool.tile([NH, 2, NG], F16)
            nc.sync.dma_start(gtp_sb[:], gtp.rearrange("e p u -> p e u"))
            bbc_sb = cpool.tile([NH, 2, NH], F16)
            nc.sync.dma_start(bbc_sb[:], bbc.rearrange("c j m -> j c m"))
            cneg_sb = cpool.tile([NH, NH], F32)
            nc.sync.dma_start(cneg_sb[:], cneg)

            for g in range(NSLICES // 4):
                # xt[p, s, 448]: cols [e*224 + j] = X_s[2p + e, j]; each
                # partition reads one contiguous 1792B run per slice
                xt = xpool.tile([NH, 4, 2 * N], F16, tag="xt")
                nc.gpsimd.dma_start(
                    xt[:],
                    x[4 * g : 4 * g + 4].rearrange("s (p e) j -> p s (e j)", e=2),
                )
                # w1t4[p, h, s, u] = W1T_s[112h + p, u] = W1_s[u, 112h + p]
                w1t4 = w1t4_pool.tile([NH, 2, 4, NG], F16)
                for q in range(2):  # slice pairs
                    w1tp = w1tpsum.tile([NH, 2, 2, NG], F32)  # [p, si, h, u]
                    for si in range(2):
                        sl = 2 * q + si
                        for h in range(2):  # W1T row chunk (j)
                            for e in range(2):  # contraction chunk (i parity)
                                nc.tensor.matmul(
                                    w1tp[:, si, h, :],
                                    xt[:, sl, e * N + h * NH : e * N + (h + 1) * NH],
                                    gtp_sb[:, e, :],
                                    start=(e == 0),
                                    stop=(e == 1),
                                )
                    nc.scalar.copy(
                        w1t4[:, :, 2 * q : 2 * q + 2, :],
                        w1tp[:].rearrange("p si h u -> p h si u"),
                    )
                v4 = vpsum.tile([NG, 4, NG], F32)
                s4 = spsum.tile([NH, 4], F32)
                for h in range(2):
                    nc.tensor.matmul(
                        v4[:],
                        gt_sb[:, h, :],
                        w1t4[:, h],
                        start=(h == 0),
                        stop=(h == 1),
                    )
                for h in range(2):
                    nc.tensor.matmul(
                        s4[:],
                        bbc_sb[:, h, :],
                        w1t4[:, h, :, NH : NH + 1],
                        start=(h == 0),
                        stop=(h == 1),
                    )
                vout = vout_pool.tile([NH, 4, NH], F32)
                for sl in range(4):
                    # vout = cneg * s + V  (fused correction + PSUM eviction)
                    nc.vector.scalar_tensor_tensor(
                        out=vout[:, sl, :],
                        in0=cneg_sb[:],
                        scalar=s4[:, sl : sl + 1],
                        in1=v4[0:NH, sl, 0:NH],
                        op0=mult,
                        op1=add,
                    )
                nc.sync.dma_start(
                    outT[4 * g : 4 * g + 4].rearrange("s v u -> v s u"), vout[:]
                )
    nc.compile()
    return nc


_CACHE: dict = {}


def _get_compiled():
    if "nc" not in _CACHE:
        _CACHE["consts"] = _build_consts()
        _CACHE["nc"] = _build_nc()
    return _CACHE["nc"], _CACHE["consts"]


def run(x: np.ndarray, trace: bool = False):
    """Returns (out [16,64,112,112] fp32, BassKernelResults)."""
    nc, (gt16, gtp16, bbc16, cneg) = _get_compiled()
    x = np.ascontiguousarray(np.asarray(x, dtype=np.float32))
    shards = x.reshape(NCORES, NSLICES, N, N)
    in_maps = [
        {"x": shards[i], "gt": gt16, "gtp": gtp16, "bbc": bbc16, "cneg": cneg}
        for i in range(NCORES)
    ]
    last_err = None
    for _attempt in range(3):
        try:
            res = run_bass_kernel_spmd(
                nc, in_maps, core_ids=list(range(NCORES)), trace=trace
            )
            break
        except Exception as e:  # transient NRT device errors: retry
            last_err = e
    else:
        raise last_err
    outT = np.stack([r["outT"] for r in res.results], axis=0)
    out = np.ascontiguousarray(
        outT.reshape(B * C, NH, NH).transpose(0, 2, 1)
    ).reshape(B, C, NH, NH)
    return out, res


def kernel(x: np.ndarray) -> np.ndarray:
    out, _ = run(x, trace=False)
    return out


# revision 8
# speedup vs baseline: 1.1761x; 1.1761x over previous
"""FLC pooling (FFT2 -> center-crop low freqs -> IFFT2, real part) on 8 trn2 cores.

Math: per (n,c) slice, out = Re(M @ X @ M.T) where M (112x224) is the 1D
fft -> fftshift -> crop -> ifftshift -> ifft operator. Im(M) is exactly
rank-1 (= outer(a, b), a[u] = a0*(-1)^u), so with R = Re(M), G = [R; b]:

    out_ext = G @ X @ G.T            (113x113; [112,112] entry = b'Xb)
    out = out_ext[:112,:112] - out_ext[112,112] * a0^2 * checkerboard

Device pipeline (fp16 operands, fp32 PSUM accumulation):
    W1T = X.T @ G.T      pass 1: stationary = X chunks (fp16), streams G.T;
                         produces the *transposed* intermediate directly,
                         so no PE transposes / identity are needed.
    V   = G @ W1T        pass 2: = out_ext^T, 4 slices batched (N=452)
    s   = b.T X b        tiny matmul against W1T col 112, broadcast to
                         all partitions via a constant-column lhsT
    vout = cneg*s + V    one fused DVE scalar_tensor_tensor per slice
Host unshard transposes each 112x112 slice (free re-layout).

x is loaded by gpsimd casting DMA (fp32 HBM -> fp16 SBUF), keeping the
Sync engine free and halving SBUF traffic; 4 slices per DMA descriptor.

Sharding: batch*channel = 1024 independent (n,c) slices -> 128 per core.
"""

import sys

sys.path.insert(0, "/opt/trn_rl_repo")

import numpy as np

import concourse.bass as bass  # noqa: F401
import concourse.mybir as mybir
import concourse.tile as tile
from concourse import bacc
from concourse.bass_utils import run_bass_kernel_spmd

N = 224
NH = 112
NG = 113  # rows of G = [R; b]
B, C = 16, 64
NCORES = 8
NSLICES = B * C // NCORES  # 128 slices per core
F32 = mybir.dt.float32
F16 = mybir.dt.float16


def _build_consts():
    F = np.fft.fft(np.eye(N), axis=0, norm="forward")
    M = np.fft.ifft(
        np.fft.ifftshift(np.fft.fftshift(F, axes=0)[N // 4 : 3 * N // 4], axes=0),
        axis=0,
        norm="forward",
    )
    R, S = M.real, M.imag
    u, sv, vt = np.linalg.svd(S)
    a = u[:, 0] * np.sqrt(sv[0])
    b = vt[0] * np.sqrt(sv[0])
    if np.abs(S - np.outer(a, b)).max() > 1e-10:
        a, b = -a, -b
    assert np.abs(S - np.outer(a, b)).max() < 1e-12
    G = np.vstack([R, b[None, :]])  # [113, 224]
    # gt16[c][i, u] = G[u, 112c + i]  (G^T row chunks, fp16; pass-2 lhsT)
    gt16 = np.ascontiguousarray(G.T.reshape(2, NH, NG)).astype(np.float16)
    # gtp16[e][p, u] = G[u, 2p + e]  (G^T rows by parity, fp16; pass-1 rhs --
    # pairs with x loaded two-adjacent-rows-per-partition)
    gtp16 = np.ascontiguousarray(
        G.T.reshape(NH, 2, NG).transpose(1, 0, 2)
    ).astype(np.float16)
    # bbc16[c][j, m] = b[112c + j] for all m (column-broadcast b)
    bbc16 = np.ascontiguousarray(
        np.repeat(b.reshape(2, NH, 1), NH, axis=2)
    ).astype(np.float16)
    a0sq = float(a[0] * a[0])  # = 1/224
    vv = np.arange(NH)
    cneg = (-a0sq * ((-1.0) ** (vv[:, None] + vv[None, :]))).astype(np.float32)
    return gt16, gtp16, bbc16, cneg


def _build_nc():
    nc = bacc.Bacc("TRN2", target_bir_lowering=False, debug=False)
    x = nc.dram_tensor("x", [NSLICES, N, N], F32, kind="ExternalInput").ap()
    gt = nc.dram_tensor("gt", [2, NH, NG], F16, kind="ExternalInput").ap()
    gtp = nc.dram_tensor("gtp", [2, NH, NG], F16, kind="ExternalInput").ap()
    bbc = nc.dram_tensor("bbc", [2, NH, NH], F16, kind="ExternalInput").ap()
    cneg = nc.dram_tensor("cneg", [NH, NH], F32, kind="ExternalInput").ap()
    outT = nc.dram_tensor("outT", [NSLICES, NH, NH], F16, kind="ExternalOutput").ap()

    mult = mybir.AluOpType.mult
    add = mybir.AluOpType.add

    with tile.TileContext(nc) as tc:
        with (
            tc.tile_pool(name="consts", bufs=1) as cpool,
            tc.tile_pool(name="xt", bufs=6) as xpool,
            tc.tile_pool(name="w1t4", bufs=4) as w1t4_pool,
            tc.tile_pool(name="vout", bufs=4) as vout_pool,
            tc.tile_pool(name="w1tp", bufs=3, space="PSUM") as w1tpsum,
            tc.tile_pool(name="v4p", bufs=2, space="PSUM") as vpsum,
            tc.tile_pool(name="s4p", bufs=2, space="PSUM") as spsum,
        ):
            gt_sb = cpool.tile([NH, 2, NG], F16)
            nc.sync.dma_start(gt_sb[:], gt.rearrange("c i u -> i c u"))
            gtp_sb = cpool.tile([NH, 2, NG], F16)
            nc.sync.dma_start(gtp_sb[:], gtp.rearrange("e p u -> p e u"))
            bbc_sb = cpool.tile([NH, 2, NH], F16)
            nc.sync.dma_start(bbc_sb[:], bbc.rearrange("c j m -> j c m"))
            cneg_sb = cpool.tile([NH, NH], F32)
            nc.sync.dma_start(cneg_sb[:], cneg)

            for g in range(NSLICES // 4):
                # xt[p, s, 448]: cols [e*224 + j] = X_s[2p + e, j]; each
                # partition reads one contiguous 1792B run per slice
                xt = xpool.tile([NH, 4, 2 * N], F16, tag="xt")
                nc.gpsimd.dma_start(
                    xt[:],
                    x[4 * g : 4 * g + 4].rearrange("s (p e) j -> p s (e j)", e=2),
                )
                # w1t4[p, h, s, u] = W1T_s[112h + p, u] = W1_s[u, 112h + p]
                w1t4 = w1t4_pool.tile([NH, 2, 4, NG], F16)
                for q in range(2):  # slice pairs
                    w1tp = w1tpsum.tile([NH, 2, 2, NG], F32)  # [p, si, h, u]
                    for si in range(2):
                        sl = 2 * q + si
                        for h in range(2):  # W1T row chunk (j)
                            for e in range(2):  # contraction chunk (i parity)
                                nc.tensor.matmul(
                                    w1tp[:, si, h, :],
                                    xt[:, sl, e * N + h * NH : e * N + (h + 1) * NH],
                                    gtp_sb[:, e, :],
                                    start=(e == 0),
                                    stop=(e == 1),
                                )
                    nc.scalar.copy(
                        w1t4[:, :, 2 * q : 2 * q + 2, :],
                        w1tp[:].rearrange("p si h u -> p h si u"),
                    )
                v4 = vpsum.tile([NG, 4, NG], F32)
                s4 = spsum.tile([NH, 4], F32)
                for h in range(2):
                    nc.tensor.matmul(
                        v4[:],
                        gt_sb[:, h, :],
                        w1t4[:, h],
                        start=(h == 0),
                        stop=(h == 1),
                    )
                for h in range(2):
                    nc.tensor.matmul(
                        s4[:],
                        bbc_sb[:, h, :],
                        w1t4[:, h, :, NH : NH + 1],
                        start=(h == 0),
                        stop=(h == 1),
                    )
                vout = vout_pool.tile([NH, 4, NH], F16)
                for sl in range(4):
                    # vout = cneg * s + V  (fused correction + PSUM eviction)
                    nc.vector.scalar_tensor_tensor(
                        out=vout[:, sl, :],
                        in0=cneg_sb[:],
                        scalar=s4[:, sl : sl + 1],
                        in1=v4[0:NH, sl, 0:NH],
                        op0=mult,
                        op1=add,
                    )
                nc.sync.dma_start(
                    outT[4 * g : 4 * g + 4].rearrange("s v u -> v s u"), vout[:]
                )
    nc.compile()
    return nc


_CACHE: dict = {}


def _get_compiled():
    if "nc" not in _CACHE:
        _CACHE["consts"] = _build_consts()
        _CACHE["nc"] = _build_nc()
    return _CACHE["nc"], _CACHE["consts"]


def run(x: np.ndarray, trace: bool = False):
    """Returns (out [16,64,112,112] fp32, BassKernelResults)."""
    nc, (gt16, gtp16, bbc16, cneg) = _get_compiled()
    x = np.ascontiguousarray(np.asarray(x, dtype=np.float32))
    shards = x.reshape(NCORES, NSLICES, N, N)
    in_maps = [
        {"x": shards[i], "gt": gt16, "gtp": gtp16, "bbc": bbc16, "cneg": cneg}
        for i in range(NCORES)
    ]
    last_err = None
    for _attempt in range(3):
        try:
            res = run_bass_kernel_spmd(
                nc, in_maps, core_ids=list(range(NCORES)), trace=trace
            )
            break
        except Exception as e:  # transient NRT device errors: retry
            last_err = e
    else:
        raise last_err
    outT = np.stack([r["outT"] for r in res.results], axis=0)
    out = np.ascontiguousarray(
        outT.reshape(B * C, NH, NH).astype(np.float32).transpose(0, 2, 1)
    ).reshape(B, C, NH, NH)
    return out, res


def kernel(x: np.ndarray) -> np.ndarray:
    out, _ = run(x, trace=False)
    return out


# revision 9
# speedup vs baseline: 1.1770x; 1.0008x over previous
"""FLC pooling (FFT2 -> center-crop low freqs -> IFFT2, real part) on 8 trn2 cores.

Math: per (n,c) slice, out = Re(M @ X @ M.T) where M (112x224) is the 1D
fft -> fftshift -> crop -> ifftshift -> ifft operator. Im(M) is exactly
rank-1 (= outer(a, b), a[u] = a0*(-1)^u), so with R = Re(M), G = [R; b]:

    out_ext = G @ X @ G.T            (113x113; [112,112] entry = b'Xb)
    out = out_ext[:112,:112] - out_ext[112,112] * a0^2 * checkerboard

Device pipeline (fp16 operands, fp32 PSUM accumulation):
    W1T = X.T @ G.T      pass 1: stationary = X chunks (fp16), streams G.T;
                         produces the *transposed* intermediate directly,
                         so no PE transposes / identity are needed.
    V   = G @ W1T        pass 2: = out_ext^T, 4 slices batched (N=452)
    s   = b.T X b        tiny matmul against W1T col 112, broadcast to
                         all partitions via a constant-column lhsT
    vout = cneg*s + V    one fused DVE scalar_tensor_tensor per slice
Host unshard transposes each 112x112 slice (free re-layout).

x is loaded by gpsimd casting DMA (fp32 HBM -> fp16 SBUF), keeping the
Sync engine free and halving SBUF traffic; 4 slices per DMA descriptor.

Sharding: batch*channel = 1024 independent (n,c) slices -> 128 per core.
"""

import sys

sys.path.insert(0, "/opt/trn_rl_repo")

import numpy as np

import concourse.bass as bass  # noqa: F401
import concourse.mybir as mybir
import concourse.tile as tile
from concourse import bacc
from concourse.bass_utils import run_bass_kernel_spmd

N = 224
NH = 112
NG = 113  # rows of G = [R; b]
B, C = 16, 64
NCORES = 8
NSLICES = B * C // NCORES  # 128 slices per core
F32 = mybir.dt.float32
F16 = mybir.dt.float16


def _build_consts():
    F = np.fft.fft(np.eye(N), axis=0, norm="forward")
    M = np.fft.ifft(
        np.fft.ifftshift(np.fft.fftshift(F, axes=0)[N // 4 : 3 * N // 4], axes=0),
        axis=0,
        norm="forward",
    )
    R, S = M.real, M.imag
    u, sv, vt = np.linalg.svd(S)
    a = u[:, 0] * np.sqrt(sv[0])
    b = vt[0] * np.sqrt(sv[0])
    if np.abs(S - np.outer(a, b)).max() > 1e-10:
        a, b = -a, -b
    assert np.abs(S - np.outer(a, b)).max() < 1e-12
    G = np.vstack([R, b[None, :]])  # [113, 224]
    # gt16[c][i, u] = G[u, 112c + i]  (G^T row chunks, fp16; pass-2 lhsT)
    gt16 = np.ascontiguousarray(G.T.reshape(2, NH, NG)).astype(np.float16)
    # gtp16[e][p, u] = G[u, 2p + e]  (G^T rows by parity, fp16; pass-1 rhs --
    # pairs with x loaded two-adjacent-rows-per-partition)
    gtp16 = np.ascontiguousarray(
        G.T.reshape(NH, 2, NG).transpose(1, 0, 2)
    ).astype(np.float16)
    # bbc16[c][j, m] = b[112c + j] for all m (column-broadcast b)
    bbc16 = np.ascontiguousarray(
        np.repeat(b.reshape(2, NH, 1), NH, axis=2)
    ).astype(np.float16)
    a0sq = float(a[0] * a[0])  # = 1/224
    vv = np.arange(NH)
    cneg = (-a0sq * ((-1.0) ** (vv[:, None] + vv[None, :]))).astype(np.float32)
    return gt16, gtp16, bbc16, cneg


def _build_nc():
    nc = bacc.Bacc("TRN2", target_bir_lowering=False, debug=False)
    x = nc.dram_tensor("x", [NSLICES, N, N], F32, kind="ExternalInput").ap()
    gt = nc.dram_tensor("gt", [2, NH, NG], F16, kind="ExternalInput").ap()
    gtp = nc.dram_tensor("gtp", [2, NH, NG], F16, kind="ExternalInput").ap()
    bbc = nc.dram_tensor("bbc", [2, NH, NH], F16, kind="ExternalInput").ap()
    cneg = nc.dram_tensor("cneg", [NH, NH], F32, kind="ExternalInput").ap()
    outT = nc.dram_tensor("outT", [NSLICES, NH, NH], F16, kind="ExternalOutput").ap()

    mult = mybir.AluOpType.mult
    add = mybir.AluOpType.add

    with tile.TileContext(nc) as tc:
        with (
            tc.tile_pool(name="consts", bufs=1) as cpool,
            tc.tile_pool(name="xt", bufs=3) as xpool,
            tc.tile_pool(name="w1t4", bufs=4) as w1t4_pool,
            tc.tile_pool(name="vout", bufs=4) as vout_pool,
            tc.tile_pool(name="w1tp", bufs=3, space="PSUM") as w1tpsum,
            tc.tile_pool(name="v4p", bufs=2, space="PSUM") as vpsum,
            tc.tile_pool(name="s4p", bufs=2, space="PSUM") as spsum,
        ):
            gt_sb = cpool.tile([NH, 2, NG], F16)
            nc.sync.dma_start(gt_sb[:], gt.rearrange("c i u -> i c u"))
            gtp_sb = cpool.tile([NH, 2, NG], F16)
            nc.sync.dma_start(gtp_sb[:], gtp.rearrange("e p u -> p e u"))
            bbc_sb = cpool.tile([NH, 2, NH], F16)
            nc.sync.dma_start(bbc_sb[:], bbc.rearrange("c j m -> j c m"))
            cneg_sb = cpool.tile([NH, NH], F32)
            nc.sync.dma_start(cneg_sb[:], cneg)

            for g in range(NSLICES // 4):
                # xt[p, s, 448]: cols [e*224 + j] = X_s[2p + e, j]; each
                # partition reads one contiguous 1792B run per slice.
                # One 1.6MB casting DMA covers two 4-slice groups.
                if g % 2 == 0:
                    xt8 = xpool.tile([NH, 8, 2 * N], F16, tag="xt")
                    nc.gpsimd.dma_start(
                        xt8[:],
                        x[4 * g : 4 * g + 8].rearrange("s (p e) j -> p s (e j)", e=2),
                    )
                xt = xt8[:, 4 * (g % 2) : 4 * (g % 2) + 4, :]
                # w1t4[p, h, s, u] = W1T_s[112h + p, u] = W1_s[u, 112h + p]
                w1t4 = w1t4_pool.tile([NH, 2, 4, NG], F16)
                for q in range(2):  # slice pairs
                    w1tp = w1tpsum.tile([NH, 2, 2, NG], F32)  # [p, si, h, u]
                    for si in range(2):
                        sl = 2 * q + si
                        for h in range(2):  # W1T row chunk (j)
                            for e in range(2):  # contraction chunk (i parity)
                                nc.tensor.matmul(
                                    w1tp[:, si, h, :],
                                    xt[:, sl, e * N + h * NH : e * N + (h + 1) * NH],
                                    gtp_sb[:, e, :],
                                    start=(e == 0),
                                    stop=(e == 1),
                                )
                    nc.scalar.copy(
                        w1t4[:, :, 2 * q : 2 * q + 2, :],
                        w1tp[:].rearrange("p si h u -> p h si u"),
                    )
                v4 = vpsum.tile([NG, 4, NG], F32)
                s4 = spsum.tile([NH, 4], F32)
                for h in range(2):
                    nc.tensor.matmul(
                        v4[:],
                        gt_sb[:, h, :],
                        w1t4[:, h],
                        start=(h == 0),
                        stop=(h == 1),
                    )
                for h in range(2):
                    nc.tensor.matmul(
                        s4[:],
                        bbc_sb[:, h, :],
                        w1t4[:, h, :, NH : NH + 1],
                        start=(h == 0),
                        stop=(h == 1),
                    )
                vout = vout_pool.tile([NH, 4, NH], F16)
                for sl in range(4):
                    # vout = cneg * s + V  (fused correction + PSUM eviction)
                    nc.vector.scalar_tensor_tensor(
                        out=vout[:, sl, :],
                        in0=cneg_sb[:],
                        scalar=s4[:, sl : sl + 1],
                        in1=v4[0:NH, sl, 0:NH],
                        op0=mult,
                        op1=add,
                    )
                nc.sync.dma_start(
                    outT[4 * g : 4 * g + 4].rearrange("s v u -> v s u"), vout[:]
                )
    nc.compile()
    return nc


_CACHE: dict = {}


def _get_compiled():
    if "nc" not in _CACHE:
        _CACHE["consts"] = _build_consts()
        _CACHE["nc"] = _build_nc()
    return _CACHE["nc"], _CACHE["consts"]


def run(x: np.ndarray, trace: bool = False):
    """Returns (out [16,64,112,112] fp32, BassKernelResults)."""
    nc, (gt16, gtp16, bbc16, cneg) = _get_compiled()
    x = np.ascontiguousarray(np.asarray(x, dtype=np.float32))
    shards = x.reshape(NCORES, NSLICES, N, N)
    in_maps = [
        {"x": shards[i], "gt": gt16, "gtp": gtp16, "bbc": bbc16, "cneg": cneg}
        for i in range(NCORES)
    ]
    last_err = None
    for _attempt in range(3):
        try:
            res = run_bass_kernel_spmd(
                nc, in_maps, core_ids=list(range(NCORES)), trace=trace
            )
            break
        except Exception as e:  # transient NRT device errors: retry
            last_err = e
    else:
        raise last_err
    outT = np.stack([r["outT"] for r in res.results], axis=0)
    out = np.ascontiguousarray(
        outT.reshape(B * C, NH, NH).astype(np.float32).transpose(0, 2, 1)
    ).reshape(B, C, NH, NH)
    return out, res


def kernel(x: np.ndarray) -> np.ndarray:
    out, _ = run(x, trace=False)
    return out


# revision 10
# speedup vs baseline: 1.1890x; 1.0102x over previous
"""FLC pooling (FFT2 -> center-crop low freqs -> IFFT2, real part) on 8 trn2 cores.

Math: per (n,c) slice, out = Re(M @ X @ M.T) where M (112x224) is the 1D
fft -> fftshift -> crop -> ifftshift -> ifft operator. Im(M) is exactly
rank-1 (= outer(a, b), a[u] = a0*(-1)^u), so with R = Re(M), G = [R; b]:

    out_ext = G @ X @ G.T            (113x113; [112,112] entry = b'Xb)
    out = out_ext[:112,:112] - out_ext[112,112] * a0^2 * checkerboard

Device pipeline (fp16 operands, fp32 PSUM accumulation):
    W1T = X.T @ G.T      pass 1: stationary = X chunks (fp16), streams G.T;
                         produces the *transposed* intermediate directly,
                         so no PE transposes / identity are needed.
    V   = G @ W1T        pass 2: = out_ext^T, 4 slices batched (N=452)
    s   = b.T X b        tiny matmul against W1T col 112, broadcast to
                         all partitions via a constant-column lhsT
    vout = cneg*s + V    one fused DVE scalar_tensor_tensor per slice
Host unshard transposes each 112x112 slice (free re-layout).

x is loaded by gpsimd casting DMA (fp32 HBM -> fp16 SBUF), keeping the
Sync engine free and halving SBUF traffic; 4 slices per DMA descriptor.

Sharding: batch*channel = 1024 independent (n,c) slices -> 128 per core.
"""

import sys

sys.path.insert(0, "/opt/trn_rl_repo")

import numpy as np

import concourse.bass as bass  # noqa: F401
import concourse.mybir as mybir
import concourse.tile as tile
from concourse import bacc
from concourse.bass_utils import run_bass_kernel_spmd

N = 224
NH = 112
NG = 113  # rows of G = [R; b]
B, C = 16, 64
NCORES = 8
NSLICES = B * C // NCORES  # 128 slices per core
F32 = mybir.dt.float32
F16 = mybir.dt.float16


def _build_consts():
    F = np.fft.fft(np.eye(N), axis=0, norm="forward")
    M = np.fft.ifft(
        np.fft.ifftshift(np.fft.fftshift(F, axes=0)[N // 4 : 3 * N // 4], axes=0),
        axis=0,
        norm="forward",
    )
    R, S = M.real, M.imag
    u, sv, vt = np.linalg.svd(S)
    a = u[:, 0] * np.sqrt(sv[0])
    b = vt[0] * np.sqrt(sv[0])
    if np.abs(S - np.outer(a, b)).max() > 1e-10:
        a, b = -a, -b
    assert np.abs(S - np.outer(a, b)).max() < 1e-12
    G = np.vstack([R, b[None, :]])  # [113, 224]
    # gt16[c][i, u] = G[u, 112c + i]  (G^T row chunks, fp16; pass-2 lhsT)
    gt16 = np.ascontiguousarray(G.T.reshape(2, NH, NG)).astype(np.float16)
    # gtp16[e][p, u] = G[u, 2p + e]  (G^T rows by parity, fp16; pass-1 rhs --
    # pairs with x loaded two-adjacent-rows-per-partition)
    gtp16 = np.ascontiguousarray(
        G.T.reshape(NH, 2, NG).transpose(1, 0, 2)
    ).astype(np.float16)
    # bbc16[c][j, m] = b[112c + j] for all m (column-broadcast b)
    bbc16 = np.ascontiguousarray(
        np.repeat(b.reshape(2, NH, 1), NH, axis=2)
    ).astype(np.float16)
    a0sq = float(a[0] * a[0])  # = 1/224
    vv = np.arange(NH)
    cneg = (-a0sq * ((-1.0) ** (vv[:, None] + vv[None, :]))).astype(np.float32)
    return gt16, gtp16, bbc16, cneg


def _build_nc():
    nc = bacc.Bacc("TRN2", target_bir_lowering=False, debug=False)
    x = nc.dram_tensor("x", [NSLICES, N, N], F32, kind="ExternalInput").ap()
    gt = nc.dram_tensor("gt", [2, NH, NG], F16, kind="ExternalInput").ap()
    gtp = nc.dram_tensor("gtp", [2, NH, NG], F16, kind="ExternalInput").ap()
    bbc = nc.dram_tensor("bbc", [2, NH, NH], F16, kind="ExternalInput").ap()
    cneg = nc.dram_tensor("cneg", [NH, NH], F32, kind="ExternalInput").ap()
    outT = nc.dram_tensor("outT", [NSLICES, NH, NH], F16, kind="ExternalOutput").ap()

    mult = mybir.AluOpType.mult
    add = mybir.AluOpType.add

    with tile.TileContext(nc) as tc:
        with (
            tc.tile_pool(name="consts", bufs=1) as cpool,
            tc.tile_pool(name="xt", bufs=8) as xpool,
            tc.tile_pool(name="w1t4", bufs=6) as w1t4_pool,
            tc.tile_pool(name="vout", bufs=6) as vout_pool,
            tc.tile_pool(name="w1tp", bufs=3, space="PSUM") as w1tpsum,
            tc.tile_pool(name="v4p", bufs=2, space="PSUM") as vpsum,
            tc.tile_pool(name="s4p", bufs=2, space="PSUM") as spsum,
        ):
            gt_sb = cpool.tile([NH, 2, NG], F16)
            nc.sync.dma_start(gt_sb[:], gt.rearrange("c i u -> i c u"))
            gtp_sb = cpool.tile([NH, 2, NG], F16)
            nc.sync.dma_start(gtp_sb[:], gtp.rearrange("e p u -> p e u"))
            bbc_sb = cpool.tile([NH, 2, NH], F16)
            nc.sync.dma_start(bbc_sb[:], bbc.rearrange("c j m -> j c m"))
            cneg_sb = cpool.tile([NH, NH], F32)
            nc.sync.dma_start(cneg_sb[:], cneg)

            for g in range(NSLICES // 4):
                # xt[p, s, 448]: cols [e*224 + j] = X_s[2p + e, j]; each
                # partition reads one contiguous 1792B run per slice.
                # One 1.6MB casting DMA covers two 4-slice groups.
                if g % 2 == 0:
                    xt8 = xpool.tile([NH, 8, 2 * N], F16, tag="xt")
                    nc.gpsimd.dma_start(
                        xt8[:],
                        x[4 * g : 4 * g + 8].rearrange("s (p e) j -> p s (e j)", e=2),
                    )
                xt = xt8[:, 4 * (g % 2) : 4 * (g % 2) + 4, :]
                # w1t4[p, h, s, u] = W1T_s[112h + p, u] = W1_s[u, 112h + p]
                w1t4 = w1t4_pool.tile([NH, 2, 4, NG], F16)
                for q in range(2):  # slice pairs
                    w1tp = w1tpsum.tile([NH, 2, 2, NG], F32)  # [p, si, h, u]
                    for si in range(2):
                        sl = 2 * q + si
                        for h in range(2):  # W1T row chunk (j)
                            for e in range(2):  # contraction chunk (i parity)
                                nc.tensor.matmul(
                                    w1tp[:, si, h, :],
                                    xt[:, sl, e * N + h * NH : e * N + (h + 1) * NH],
                                    gtp_sb[:, e, :],
                                    start=(e == 0),
                                    stop=(e == 1),
                                )
                    nc.scalar.copy(
                        w1t4[:, :, 2 * q : 2 * q + 2, :],
                        w1tp[:].rearrange("p si h u -> p h si u"),
                    )
                v4 = vpsum.tile([NG, 4, NG], F32)
                s4 = spsum.tile([NH, 4], F32)
                for h in range(2):
                    nc.tensor.matmul(
                        v4[:],
                        gt_sb[:, h, :],
                        w1t4[:, h],
                        start=(h == 0),
                        stop=(h == 1),
                    )
                for h in range(2):
                    nc.tensor.matmul(
                        s4[:],
                        bbc_sb[:, h, :],
                        w1t4[:, h, :, NH : NH + 1],
                        start=(h == 0),
                        stop=(h == 1),
                    )
                vout = vout_pool.tile([NH, 4, NH], F16)
                for sl in range(4):
                    # vout = cneg * s + V  (fused correction + PSUM eviction)
                    nc.vector.scalar_tensor_tensor(
                        out=vout[:, sl, :],
                        in0=cneg_sb[:],
                        scalar=s4[:, sl : sl + 1],
                        in1=v4[0:NH, sl, 0:NH],
                        op0=mult,
                        op1=add,
                    )
                nc.sync.dma_start(
                    outT[4 * g : 4 * g + 4].rearrange("s v u -> v s u"), vout[:]
                )
    nc.compile()
    return nc


_CACHE: dict = {}


def _get_compiled():
    if "nc" not in _CACHE:
        _CACHE["consts"] = _build_consts()
        _CACHE["nc"] = _build_nc()
    return _CACHE["nc"], _CACHE["consts"]


def run(x: np.ndarray, trace: bool = False):
    """Returns (out [16,64,112,112] fp32, BassKernelResults)."""
    nc, (gt16, gtp16, bbc16, cneg) = _get_compiled()
    x = np.ascontiguousarray(np.asarray(x, dtype=np.float32))
    shards = x.reshape(NCORES, NSLICES, N, N)
    in_maps = [
        {"x": shards[i], "gt": gt16, "gtp": gtp16, "bbc": bbc16, "cneg": cneg}
        for i in range(NCORES)
    ]
    last_err = None
    for _attempt in range(3):
        try:
            res = run_bass_kernel_spmd(
                nc, in_maps, core_ids=list(range(NCORES)), trace=trace
            )
            break
        except Exception as e:  # transient NRT device errors: retry
            last_err = e
    else:
        raise last_err
    outT = np.stack([r["outT"] for r in res.results], axis=0)
    out = np.ascontiguousarray(
        outT.reshape(B * C, NH, NH).astype(np.float32).transpose(0, 2, 1)
    ).reshape(B, C, NH, NH)
    return out, res


def kernel(x: np.ndarray) -> np.ndarray:
    out, _ = run(x, trace=False)
    return out
